# revision 17
# baseline (speedup 1.0000x reference)
"""Multi-head attention (B=4, L=2048, E=1024, H=16, causal) for 8 Trainium2
NeuronCores.

Sharding: data-parallel over batch (4) x tensor-parallel over heads (2 groups
of 8 heads).  Core c handles batch c//2, head-group c%2.  Each core runs the
q/k/v projections for its 8 heads (column shards of wq/wk/wv), causal
flash-style attention, and its row-shard of the output projection; the
all-reduce over the two head-groups is the final gather (host-side add).

v3 schedule: few large input DMAs (the hardware DMA rings hold only ~8
outstanding descriptors — a larger t0 burst loses doorbells and stalls the
ring), PE warm-up matmuls during the initial load, group-interleaved pair
order (all q<1024 attention groups first) so the output projection weaves
into the last pairs as PE filler, projections popped as fillers on every
attention step, softmax divide copies ctx out of PSUM immediately to free
the accumulator bank, fp16 output.
"""

import numpy as np

import concourse.bass as bass
import concourse.mybir as mybir
import concourse.tile as tile
from concourse import bacc
from concourse.bass_utils import run_bass_kernel_spmd

# ---------------------------------------------------------------------------
# Problem constants (hardcoded per the harness contract)
# ---------------------------------------------------------------------------
B, L, E, H = 4, 2048, 1024, 16
DK = E // H          # 64
NCORES = 8
HL = H // 2          # heads per core = 8
DQ = HL * DK         # 512 = per-core projection width
P = 128
EC = E // P          # 8 contraction chunks
NLT = L // 512       # 4 l-tiles of 512
NKC = L // P         # 16 k chunks of 128
NDC = DQ // P        # 4 dq chunks (head pairs)
F16 = mybir.dt.float16
F32 = mybir.dt.float32
CB = 1792            # const blob cols: mask 2*128 | bvb 512 | bob 1024

_BUILT = {}


def _build(causal: bool):
    nc = bacc.Bacc("TRN2", num_devices=NCORES, debug=False)

    qT = nc.dram_tensor("qT", [P, NLT, EC, 512], F16, kind="ExternalInput")
    kT = nc.dram_tensor("kT", [P, NLT, EC, 512], F16, kind="ExternalInput")
    vT = nc.dram_tensor("vT", [P, NKC, EC, P], F16, kind="ExternalInput")
    wqT = nc.dram_tensor("wqT", [P, EC, DQ], F16, kind="ExternalInput")
    wkT = nc.dram_tensor("wkT", [P, EC, DQ], F16, kind="ExternalInput")
    wvT = nc.dram_tensor("wvT", [P, EC, DQ], F16, kind="ExternalInput")
    woT = nc.dram_tensor("woT", [P, NDC, E], F16, kind="ExternalInput")
    cb16 = nc.dram_tensor("cb16", [P, CB], F16, kind="ExternalInput")
    cb32 = nc.dram_tensor("cb32", [P, 2 * NDC], F32, kind="ExternalInput")
    out = nc.dram_tensor("out", [L, E], F16, kind="ExternalOutput")

    with tile.TileContext(nc) as tc:
        with (
            tc.tile_pool(name="const", bufs=1) as const,
            tc.tile_pool(name="persist", bufs=1) as persist,
            tc.tile_pool(name="ktp", bufs=1) as ktp,
            tc.tile_pool(name="vtp", bufs=4) as vtp,
            tc.tile_pool(name="pt", bufs=8) as ptp,
            tc.tile_pool(name="small", bufs=4) as small,
            tc.tile_pool(name="osb", bufs=2) as osb,
            tc.tile_pool(name="ps", bufs=2, space="PSUM") as psp,
            tc.tile_pool(name="dscratch", bufs=4, space="DRAM") as dsp,
        ):
            # ---- tiles -----------------------------------------------------
            cb_sb = const.tile([P, CB], F16, tag="cb")
            cb32_sb = const.tile([P, 2 * NDC], F32, tag="cb32")
            mask_sb = cb_sb[:, 0:P]                       # [P, 128]
            mask2_sb = cb_sb[:, 0 : 2 * P]                # warm-up operand
            bvb_sb = cb_sb[:, 2 * P : 2 * P + DQ]         # [P, 512]
            bob_sb = cb_sb[:, 2 * P + DQ : CB]            # [P, 1024]
            bq_sb = cb32_sb[:, 0:NDC]
            bk_sb = cb32_sb[:, NDC : 2 * NDC]
            wv_sb = const.tile([P, EC, DQ], F16, tag="wv")
            wk_sb = const.tile([P, EC, DQ], F16, tag="wk")
            wq_sb = const.tile([P, EC, DQ], F16, tag="wq")
            wo_sb = const.tile([P, NDC, E], F16, tag="wo")
            qf = [
                persist.tile([P, EC, 512], F16, tag=f"qf{lt}", name=f"qf_{lt}")
                for lt in range(NLT)
            ]

            QT_sb = persist.tile([P, NDC, L], F16, tag="QT")
            KT_sb = persist.tile([P, NDC, L], F16, tag="KT")
            CT_sb = persist.tile([P, NDC, L], F16, tag="CT")   # ctx^T normalized
            V_sb = persist.tile([P, NKC, HL, DK + 1], F16, tag="V")

            # ---- t0 DMA: small descriptors in dependency order, staying
            # well under the HWDGE ring depth on both rings.  Late-consumed
            # tensors (qf2/qf3, kt2/kt3, wo) are issued at emission points so
            # every transfer lands tens of us before its consumer.
            # sync ring: V path first, then consts
            nc.sync.dma_start(wv_sb[:], wvT[:])
            vt_pre = []
            for lc in range(4):
                vt = vtp.tile([P, EC, P], F16, tag="vstage", name=f"vt_{lc}")
                nc.sync.dma_start(vt[:], vT[:, lc])
                vt_pre.append(vt)
            nc.sync.dma_start(cb_sb[:], cb16[:])
            nc.sync.dma_start(cb32_sb[:], cb32[:])
            # scalar ring: K path, then wq + early Q tiles
            nc.scalar.dma_start(wk_sb[:], wkT[:])
            kt_tiles = {}
            for lt in (0, 1):
                kt_tiles[lt] = ktp.tile(
                    [P, EC, 512], F16, tag="kt", bufs=2, name=f"kt_{lt}"
                )
                nc.scalar.dma_start(kt_tiles[lt][:], kT[:, lt])
            nc.scalar.dma_start(wq_sb[:], wqT[:])
            nc.scalar.dma_start(qf[0][:], qT[:, 0])
            nc.scalar.dma_start(qf[1][:], qT[:, 1])

            junk = const.tile([P, 512], F16, tag="junk")
            nc.vector.memset(junk[:], 0.125)
            nc.vector.memset(V_sb[:, :, :, DK], 1.0)

            def qTf(lt):
                return qf[lt]

            # ---- PE warm-up: junk matmuls so the HAM clock gate reaches
            # 2.4 GHz before real work arrives (~12us of coverage).
            warm_ps = psp.tile([P, 1024], F32, tag="st", name="warm")
            for i in range(30):
                nc.tensor.matmul(
                    warm_ps[:, :512],
                    junk[:, 0:P],
                    junk[:],
                    start=True,
                    stop=True,
                )

            # ---- projection emitters --------------------------------------
            def emit_vproj(lc):
                if lc < 4:
                    vt = vt_pre[lc]
                else:
                    vt = vtp.tile([P, EC, P], F16, tag="vstage", name=f"vt_{lc}")
                    nc.sync.dma_start(vt[:], vT[:, lc])
                ps = psp.tile([P, 1024], F32, tag="st", name=f"vps_{lc}")
                for ec in range(EC):
                    nc.tensor.matmul(
                        ps[:, :512],
                        vt[:, ec, :],
                        wv_sb[:, ec, :],
                        start=(ec == 0),
                        stop=(ec == EC - 1),
                    )
                nc.vector.tensor_add(
                    V_sb[:, lc, :, 0:DK],
                    ps[:, :512].rearrange("p (h d) -> p h d", h=HL),
                    bvb_sb.rearrange("p (h d) -> p h d", h=HL),
                )

            def emit_kproj(lt, dc):
                xt = kt_tiles[lt]
                ps = psp.tile([P, 1024], F32, tag="st", name=f"kps_{dc}_{lt}")
                for ec in range(EC):
                    nc.tensor.matmul(
                        ps[:, :512],
                        wk_sb[:, ec, dc * P : (dc + 1) * P],
                        xt[:, ec, :],
                        start=(ec == 0),
                        stop=(ec == EC - 1),
                    )
                nc.vector.tensor_scalar_add(
                    KT_sb[:, dc, lt * 512 : (lt + 1) * 512],
                    ps[:, :512],
                    bk_sb[:, dc : dc + 1],
                )

            def emit_qproj(dc, lt):
                ps = psp.tile([P, 1024], F32, tag="st", name=f"qps_{dc}_{lt}")
                for ec in range(EC):
                    nc.tensor.matmul(
                        ps[:, :512],
                        wq_sb[:, ec, dc * P : (dc + 1) * P],
                        qTf(lt)[:, ec, :],
                        start=(ec == 0),
                        stop=(ec == EC - 1),
                    )
                nc.vector.tensor_scalar_add(
                    QT_sb[:, dc, lt * 512 : (lt + 1) * 512],
                    ps[:, :512],
                    bq_sb[:, dc : dc + 1],
                )

            def emit_oproj(qc, eh, defer=False):
                ps = psp.tile([P, 1024], F32, tag="st", name=f"ops_{qc}_{eh}")
                for dc in range(NDC):
                    nc.tensor.matmul(
                        ps[:, :512],
                        CT_sb[:, dc, qc * P : (qc + 1) * P],
                        wo_sb[:, dc, eh * 512 : (eh + 1) * 512],
                        start=(dc == 0),
                        stop=(dc == NDC - 1),
                    )
                ot = osb.tile([P, 512], F16, tag="ot", bufs=2, name=f"ot_{qc}_{eh}")
                nc.vector.tensor_add(
                    ot[:], ps[:, :512], bob_sb[:, eh * 512 : (eh + 1) * 512]
                )
                nc.sync.dma_start(
                    out[qc * P : (qc + 1) * P, eh * 512 : (eh + 1) * 512], ot[:]
                )

            # ---- pre-attention: V k<512, K l<1024, Q dc0 l<1024 -----------
            # (V4-7 lead the first pair's fillers: PV only touches chunk kj
            # at step kj+1, so they land in time and attention starts sooner)
            for lc in range(4):
                emit_vproj(lc)
            # late Q tiles: issued now so they land long before their
            # consumers (p3g0/p0g1 fillers)
            nc.sync.dma_start(qf[2][:], qT[:, 2])
            nc.sync.dma_start(qf[3][:], qT[:, 3])
            for dc in range(NDC):
                emit_kproj(0, dc)
            # kt0's slot is consumed; stream kt2/kt3 (consumed ~30us later)
            kt_tiles[2] = ktp.tile(
                [P, EC, 512], F16, tag="kt", bufs=2, name="kt_2"
            )
            nc.scalar.dma_start(kt_tiles[2][:], kT[:, 2])
            for dc in range(NDC):
                emit_kproj(1, dc)
            kt_tiles[3] = ktp.tile(
                [P, EC, 512], F16, tag="kt", bufs=2, name="kt_3"
            )
            nc.scalar.dma_start(kt_tiles[3][:], kT[:, 3])
            nc.scalar.dma_start(wo_sb[:], woT[:])
            emit_qproj(0, 0)
            emit_qproj(0, 1)

            # ---- filler schedule for the attention pair loops -------------
            # One unit pops per kj step.  Order respects data dependencies:
            # K lt2/3 and Q lt2/3 finish before the g1 pass touches them,
            # V8-15 before g1's PV reads k>=1024, out-projection for q<1024
            # unlocks after the whole g0 pass.
            fillers = {
                (0, 0): [lambda lc=lc: emit_vproj(lc) for lc in (4, 5, 6, 7)]
                + [lambda dc=dc: emit_kproj(2, dc) for dc in range(NDC)]
                + [lambda: emit_qproj(1, 0), lambda: emit_qproj(1, 1)]
                + [lambda lc=lc: emit_vproj(lc) for lc in (8, 9)],
                (1, 0): [lambda dc=dc: emit_kproj(3, dc) for dc in range(NDC)]
                + [lambda: emit_qproj(2, 0), lambda: emit_qproj(2, 1)]
                + [lambda lc=lc: emit_vproj(lc) for lc in (10, 11)],
                (2, 0): [lambda: emit_qproj(3, 0), lambda: emit_qproj(3, 1)]
                + [lambda lc=lc: emit_vproj(lc) for lc in (12, 13)],
                (3, 0): [lambda: emit_qproj(0, 2), lambda: emit_qproj(0, 3)]
                + [lambda lc=lc: emit_vproj(lc) for lc in (14, 15)],
                (0, 1): [
                    lambda dc=dc, lt=lt: emit_qproj(dc, lt)
                    for dc in (1, 2, 3)
                    for lt in (2, 3)
                ],
                (1, 1): [
                    lambda qc=qc, eh=eh: emit_oproj(qc, eh)
                    for qc in (0, 1, 2, 3)
                    for eh in (0, 1)
                ],
                (2, 1): [
                    lambda qc=qc, eh=eh: emit_oproj(qc, eh)
                    for qc in (4, 5, 6, 7)
                    for eh in (0, 1)
                ],
                (3, 1): [],
            }

            # ---- attention: group-interleaved pair order ------------------
            scale = float(1.0 / np.sqrt(DK))
            pending_fin = []
            for grp in range(2):
                for hp in range(NDC):
                    heads = (2 * hp, 2 * hp + 1)
                    filler = fillers[(hp, grp)]
                    glo, ghi = grp * 1024, grp * 1024 + 1024
                    qis = (2 * grp, 2 * grp + 1)
                    n_kj = (8 * grp + 8) if causal else NKC
                    ctx_ps = {
                        (h, qi): psp.tile(
                            [DK + 1, 512],
                            F32,
                            tag="ctx",
                            bufs=4,
                            name=f"ctx_{h}_{grp}_{qi}",
                        )
                        for h in heads
                        for qi in qis
                    }

                    def emit_divide(h, qi):
                        # copy ctx+den out of PSUM at once (frees the bank
                        # for the next group) and launch the denominator
                        # broadcast (DRAM bounce).  The reciprocal+multiply
                        # are DEFERRED several kj steps so the DVE never
                        # queues behind the in-flight bounce.
                        g0 = 64 * (h % 2)
                        cps = ctx_ps[(h, qi)]
                        cpy = small.tile(
                            [DK + 1, 512], F32, tag="cpy", bufs=6,
                            name=f"cpy_{h}_{qi}",
                        )
                        nc.vector.tensor_copy(cpy[:], cps[:])
                        dsc = dsp.tile([1, 512], F32, name=f"dsc_{h}_{qi}", tag="dsc")
                        nc.sync.dma_start(dsc[:], cpy[DK : DK + 1, :])
                        den64 = small.tile(
                            [64, 512], F32, tag="den64", bufs=5,
                            name=f"d64_{h}_{qi}",
                        )
                        nc.sync.dma_start(
                            den64[:], dsc[0:1, :].to_broadcast((64, 512))
                        )
                        hp_ = hp
                        qs = slice(qi * 512, (qi + 1) * 512)

                        def finalize(h=h, qi=qi, hp=hp_, qs=qs, cpy=cpy, den64=den64):
                            g0 = 64 * (h % 2)
                            rec64 = small.tile(
                                [64, 512], F32, tag="rec64", bufs=2,
                                name=f"r64_{h}_{qi}",
                            )
                            nc.vector.reciprocal_approx_fast(rec64[:], den64[:])
                            if g0 == 0:
                                nc.vector.tensor_mul(
                                    CT_sb[0:64, hp, qs], cpy[0:DK, :], rec64[:]
                                )
                            else:
                                tmp = small.tile(
                                    [64, 512], F16, tag="ctmp", bufs=2,
                                    name=f"tmp_{h}_{qi}",
                                )
                                nc.vector.tensor_mul(tmp[:], cpy[0:DK, :], rec64[:])
                                nc.sync.dma_start(CT_sb[64:128, hp, qs], tmp[:])

                        pending_fin.append(finalize)

                    def emit_st(kj):
                        q0 = max(glo, kj * P) if causal else glo
                        W = ghi - q0
                        base = (q0 // 512) * 512
                        pts = {}
                        for h in heads:
                            g0 = 64 * (h % 2)
                            st = psp.tile(
                                [P, 1024], F32, tag="st", name=f"st_{h}_{grp}_{kj}"
                            )
                            j = q0
                            while j < ghi:
                                w = min(512 - (j % 512), ghi - j)
                                nc.tensor.matmul(
                                    st[:, j - base : j - base + w],
                                    KT_sb[g0 : g0 + 64, hp, kj * P : (kj + 1) * P],
                                    QT_sb[g0 : g0 + 64, hp, j : j + w],
                                    start=True,
                                    stop=True,
                                )
                                j += w
                            pt = ptp.tile(
                                [P, 1024], F16, tag="pt", bufs=8, name=f"pt_{h}_{grp}_{kj}"
                            )
                            nc.scalar.activation(
                                pt[:, q0 - base : q0 - base + W],
                                st[:, q0 - base : q0 - base + W],
                                mybir.ActivationFunctionType.Exp,
                                scale=scale,
                            )
                            if causal and kj * P >= glo:
                                nc.vector.tensor_mul(
                                    pt[:, q0 - base : q0 - base + P],
                                    pt[:, q0 - base : q0 - base + P],
                                    mask_sb,
                                )
                            pts[h] = pt
                        return pts, q0, base

                    def emit_pv(kj, pts):
                        ptd, q0, base = pts
                        for h in heads:
                            pt = ptd[h]
                            for qi in qis:
                                lo = max(qi * 512, q0)
                                w = qi * 512 + 512 - lo
                                if w <= 0:
                                    continue
                                last = (4 * qi + 3) if causal else (NKC - 1)
                                if kj > last:
                                    continue
                                nc.tensor.matmul(
                                    ctx_ps[(h, qi)][:, lo - qi * 512 :],
                                    V_sb[:, kj, h, :],
                                    pt[:, lo - base : lo - base + w],
                                    start=(kj == 0),
                                    stop=(kj == last),
                                )
                                if kj == last:
                                    emit_divide(h, qi)

                    prev = None
                    for kj in range(n_kj):
                        pts = emit_st(kj)
                        if prev is not None:
                            emit_pv(prev[0], prev[1])
                        # finalize an older divide (its bounce has landed)
                        if len(pending_fin) > 2:
                            pending_fin.pop(0)()
                        if filler:
                            filler.pop(0)()
                        prev = (kj, pts)
                    emit_pv(prev[0], prev[1])
                    while filler:
                        filler.pop(0)()

            # flush remaining divide finalizations before the tail reads CT
            while pending_fin:
                pending_fin.pop(0)()

            # ---- output projection tail: q >= 1024 ------------------------
            for qc in range(8, L // P):
                for eh in range(E // 512):
                    emit_oproj(qc, eh)

    nc.compile()
    return nc


def _chunked(x, inner):
    """[outer*inner, n] -> [inner, outer, n] with element [p, c, n] = x[c*inner+p, n]."""
    o = x.shape[0] // inner
    return np.ascontiguousarray(x.reshape(o, inner, *x.shape[1:]).transpose(1, 0, 2))


def _lchunked(x, w):
    """activation [L, E] -> [128, L//w, 8, w] with [p, c, ec, j] =
    x[c*w+j, ec*128+p] (contiguous per-partition staging chunks)."""
    xt = x.T.astype(np.float16)                    # [E, L]
    a = xt.reshape(EC, P, L // w, w)               # [ec, p, c, j]
    return np.ascontiguousarray(a.transpose(1, 2, 0, 3))


def kernel(query, key, value, wq, bq, wk, bk, wv, bv, wo, bo, is_causal):
    return _run(query, key, value, wq, bq, wk, bk, wv, bv, wo, bo, is_causal)[0]


def _run(query, key, value, wq, bq, wk, bk, wv, bv, wo, bo, is_causal, **run_kwargs):
    query = np.asarray(query, dtype=np.float32)
    key = np.asarray(key, dtype=np.float32)
    value = np.asarray(value, dtype=np.float32)
    wq, wk, wv, wo = (np.asarray(w, dtype=np.float32) for w in (wq, wk, wv, wo))
    bq, bk, bv, bo = (np.asarray(b, dtype=np.float32) for b in (bq, bk, bv, bo))
    causal = bool(int(np.asarray(is_causal)))

    if causal not in _BUILT:
        _BUILT[causal] = _build(causal)
    nc = _BUILT[causal]

    # triangular diagonal mask: mask[kp, qf] = 1 if qf >= kp
    qf = np.arange(P)[None, :]
    kp = np.arange(P)[:, None]
    m1 = (qf >= kp).astype(np.float16)                        # [128, 128]

    in_maps = []
    for c in range(NCORES):
        b = c // 2
        g = c % 2
        hs = slice(g * DQ, (g + 1) * DQ)
        cb = np.zeros((P, CB), dtype=np.float16)
        cb[:, 0:P] = m1
        cb[:, P : 2 * P] = m1
        cb[:, 2 * P : 2 * P + DQ] = np.broadcast_to(
            bv[hs].astype(np.float16), (P, DQ)
        )
        cb[:, 2 * P + DQ : CB] = np.broadcast_to(
            (bo if g == 0 else np.zeros_like(bo)).astype(np.float16), (P, E)
        )
        c32 = np.zeros((P, 2 * NDC), dtype=np.float32)
        c32[:, 0:NDC] = bq[hs].reshape(NDC, P).T
        c32[:, NDC : 2 * NDC] = bk[hs].reshape(NDC, P).T
        m = {
            "qT": _lchunked(query[b], 512),
            "kT": _lchunked(key[b], 512),
            "vT": _lchunked(value[b], P),
            "wqT": _chunked(wq[hs, :].T.astype(np.float16), P),
            "wkT": _chunked(wk[hs, :].T.astype(np.float16), P),
            "wvT": _chunked(wv[hs, :].T.astype(np.float16), P),
            "woT": _chunked(wo[:, hs].T.astype(np.float16), P),
            "cb16": cb,
            "cb32": np.ascontiguousarray(c32),
        }
        in_maps.append(m)

    res = run_bass_kernel_spmd(nc, in_maps, core_ids=list(range(NCORES)), **run_kwargs)

    out = np.empty((B, L, E), dtype=np.float32)
    for b in range(B):
        out[b] = res.results[2 * b]["out"].astype(np.float32) + res.results[
            2 * b + 1
        ]["out"].astype(np.float32)
    return out, res


# revision 18
# speedup vs baseline: 1.0501x; 1.0501x over previous
"""Multi-head attention (B=4, L=2048, E=1024, H=16, causal) for 8 Trainium2
NeuronCores.

Sharding: data-parallel over batch (4) x tensor-parallel over heads (2 groups
of 8 heads).  Core c handles batch c//2, head-group c%2.  Each core runs the
q/k/v projections for its 8 heads (column shards of wq/wk/wv), causal
flash-style attention, and its row-shard of the output projection; the
all-reduce over the two head-groups is the final gather (host-side add).

v3 schedule: few large input DMAs (the hardware DMA rings hold only ~8
outstanding descriptors — a larger t0 burst loses doorbells and stalls the
ring), PE warm-up matmuls during the initial load, group-interleaved pair
order (all q<1024 attention groups first) so the output projection weaves
into the last pairs as PE filler, projections popped as fillers on every
attention step, softmax divide copies ctx out of PSUM immediately to free
the accumulator bank, fp16 output.
"""

import numpy as np

import concourse.bass as bass
import concourse.mybir as mybir
import concourse.tile as tile
from concourse import bacc
from concourse.bass_utils import run_bass_kernel_spmd

# ---------------------------------------------------------------------------
# Problem constants (hardcoded per the harness contract)
# ---------------------------------------------------------------------------
B, L, E, H = 4, 2048, 1024, 16
DK = E // H          # 64
NCORES = 8
HL = H // 2          # heads per core = 8
DQ = HL * DK         # 512 = per-core projection width
P = 128
EC = E // P          # 8 contraction chunks
NLT = L // 512       # 4 l-tiles of 512
NKC = L // P         # 16 k chunks of 128
NDC = DQ // P        # 4 dq chunks (head pairs)
F16 = mybir.dt.float16
F32 = mybir.dt.float32
CB = 1792            # const blob cols: mask 2*128 | bvb 512 | bob 1024

_BUILT = {}


def _build(causal: bool):
    nc = bacc.Bacc("TRN2", num_devices=NCORES, debug=False)

    qT = nc.dram_tensor("qT", [P, NLT, EC, 512], F16, kind="ExternalInput")
    kT = nc.dram_tensor("kT", [P, NLT, EC, 512], F16, kind="ExternalInput")
    vT = nc.dram_tensor("vT", [P, NKC, EC, P], F16, kind="ExternalInput")
    wqT = nc.dram_tensor("wqT", [P, EC, DQ], F16, kind="ExternalInput")
    wkT = nc.dram_tensor("wkT", [P, EC, DQ], F16, kind="ExternalInput")
    wvT = nc.dram_tensor("wvT", [P, EC, DQ], F16, kind="ExternalInput")
    woT = nc.dram_tensor("woT", [P, NDC, E], F16, kind="ExternalInput")
    cb16 = nc.dram_tensor("cb16", [P, CB], F16, kind="ExternalInput")
    cb32 = nc.dram_tensor("cb32", [P, 2 * NDC], F32, kind="ExternalInput")
    out = nc.dram_tensor("out", [L, E], F16, kind="ExternalOutput")

    with tile.TileContext(nc) as tc:
        with (
            tc.tile_pool(name="const", bufs=1) as const,
            tc.tile_pool(name="persist", bufs=1) as persist,
            tc.tile_pool(name="ktp", bufs=1) as ktp,
            tc.tile_pool(name="vtp", bufs=4) as vtp,
            tc.tile_pool(name="pt", bufs=6) as ptp,
            tc.tile_pool(name="small", bufs=4) as small,
            tc.tile_pool(name="osb", bufs=2) as osb,
            tc.tile_pool(name="ps", bufs=2, space="PSUM") as psp,
            tc.tile_pool(name="dscratch", bufs=4, space="DRAM") as dsp,
        ):
            # ---- tiles -----------------------------------------------------
            cb_sb = const.tile([P, CB], F16, tag="cb")
            cb32_sb = const.tile([P, 2 * NDC], F32, tag="cb32")
            mask_sb = cb_sb[:, 0:P]                       # [P, 128]
            mask2_sb = cb_sb[:, 0 : 2 * P]                # warm-up operand
            bvb_sb = cb_sb[:, 2 * P : 2 * P + DQ]         # [P, 512]
            bob_sb = cb_sb[:, 2 * P + DQ : CB]            # [P, 1024]
            bq_sb = cb32_sb[:, 0:NDC]
            bk_sb = cb32_sb[:, NDC : 2 * NDC]
            wv_sb = const.tile([P, EC, DQ], F16, tag="wv")
            wk_sb = const.tile([P, EC, DQ], F16, tag="wk")
            wq_sb = const.tile([P, EC, DQ], F16, tag="wq")
            wo_sb = const.tile([P, NDC, E], F16, tag="wo")
            qf = [
                persist.tile([P, EC, 512], F16, tag=f"qf{lt}", name=f"qf_{lt}")
                for lt in range(NLT)
            ]

            QT_sb = persist.tile([P, NDC, L], F16, tag="QT")
            KT_sb = persist.tile([P, NDC, L], F16, tag="KT")
            CT_sb = persist.tile([P, NDC, L], F16, tag="CT")   # ctx^T normalized
            V_sb = persist.tile([P, NKC, HL, DK + 1], F16, tag="V")

            # ---- t0 DMA: small descriptors in dependency order, staying
            # well under the HWDGE ring depth on both rings.  Late-consumed
            # tensors (qf2/qf3, kt2/kt3, wo) are issued at emission points so
            # every transfer lands tens of us before its consumer.
            # sync ring: V path first, then consts
            nc.sync.dma_start(wv_sb[:], wvT[:])
            vt_pre = []
            for lc in range(4):
                vt = vtp.tile([P, EC, P], F16, tag="vstage", name=f"vt_{lc}")
                nc.sync.dma_start(vt[:], vT[:, lc])
                vt_pre.append(vt)
            nc.sync.dma_start(cb_sb[:], cb16[:])
            nc.sync.dma_start(cb32_sb[:], cb32[:])
            # scalar ring: K path, then wq + early Q tiles
            nc.scalar.dma_start(wk_sb[:], wkT[:])
            kt_tiles = {}
            for lt in (0, 1):
                kt_tiles[lt] = ktp.tile(
                    [P, EC, 512], F16, tag="kt", bufs=2, name=f"kt_{lt}"
                )
                nc.scalar.dma_start(kt_tiles[lt][:], kT[:, lt])
            nc.scalar.dma_start(wq_sb[:], wqT[:])
            nc.scalar.dma_start(qf[0][:], qT[:, 0])
            nc.scalar.dma_start(qf[1][:], qT[:, 1])

            junk = const.tile([P, 512], F16, tag="junk")
            nc.vector.memset(junk[:], 0.125)
            nc.vector.memset(V_sb[:, :, :, DK], 1.0)

            def qTf(lt):
                return qf[lt]

            # ---- PE warm-up: junk matmuls so the HAM clock gate reaches
            # 2.4 GHz before real work arrives (~12us of coverage).
            warm_ps = psp.tile([P, 1024], F32, tag="st", name="warm")
            for i in range(30):
                nc.tensor.matmul(
                    warm_ps[:, :512],
                    junk[:, 0:P],
                    junk[:],
                    start=True,
                    stop=True,
                )

            # ---- projection emitters --------------------------------------
            def emit_vproj(lc):
                if lc < 4:
                    vt = vt_pre[lc]
                else:
                    vt = vtp.tile([P, EC, P], F16, tag="vstage", name=f"vt_{lc}")
                    nc.sync.dma_start(vt[:], vT[:, lc])
                ps = psp.tile([P, 1024], F32, tag="st", name=f"vps_{lc}")
                for ec in range(EC):
                    nc.tensor.matmul(
                        ps[:, :512],
                        vt[:, ec, :],
                        wv_sb[:, ec, :],
                        start=(ec == 0),
                        stop=(ec == EC - 1),
                    )
                nc.vector.tensor_add(
                    V_sb[:, lc, :, 0:DK],
                    ps[:, :512].rearrange("p (h d) -> p h d", h=HL),
                    bvb_sb.rearrange("p (h d) -> p h d", h=HL),
                )

            def emit_kproj(lt, dc):
                xt = kt_tiles[lt]
                ps = psp.tile([P, 1024], F32, tag="st", name=f"kps_{dc}_{lt}")
                for ec in range(EC):
                    nc.tensor.matmul(
                        ps[:, :512],
                        wk_sb[:, ec, dc * P : (dc + 1) * P],
                        xt[:, ec, :],
                        start=(ec == 0),
                        stop=(ec == EC - 1),
                    )
                nc.vector.tensor_scalar_add(
                    KT_sb[:, dc, lt * 512 : (lt + 1) * 512],
                    ps[:, :512],
                    bk_sb[:, dc : dc + 1],
                )

            def emit_qproj(dc, lt):
                ps = psp.tile([P, 1024], F32, tag="st", name=f"qps_{dc}_{lt}")
                for ec in range(EC):
                    nc.tensor.matmul(
                        ps[:, :512],
                        wq_sb[:, ec, dc * P : (dc + 1) * P],
                        qTf(lt)[:, ec, :],
                        start=(ec == 0),
                        stop=(ec == EC - 1),
                    )
                nc.vector.tensor_scalar_add(
                    QT_sb[:, dc, lt * 512 : (lt + 1) * 512],
                    ps[:, :512],
                    bq_sb[:, dc : dc + 1],
                )

            def emit_oproj(qc, eh, defer=False):
                ps = psp.tile([P, 1024], F32, tag="st", name=f"ops_{qc}_{eh}")
                for dc in range(NDC):
                    nc.tensor.matmul(
                        ps[:, :512],
                        CT_sb[:, dc, qc * P : (qc + 1) * P],
                        wo_sb[:, dc, eh * 512 : (eh + 1) * 512],
                        start=(dc == 0),
                        stop=(dc == NDC - 1),
                    )
                ot = osb.tile([P, 512], F16, tag="ot", bufs=2, name=f"ot_{qc}_{eh}")
                nc.vector.tensor_add(
                    ot[:], ps[:, :512], bob_sb[:, eh * 512 : (eh + 1) * 512]
                )
                eng = nc.sync if (2 * qc + eh) % 2 == 0 else nc.scalar
                eng.dma_start(
                    out[qc * P : (qc + 1) * P, eh * 512 : (eh + 1) * 512], ot[:]
                )

            # ---- pre-attention: V k<1024, K l<1024, Q dc0 l<1024 ----------
            for lc in range(8):
                emit_vproj(lc)
            # late Q tiles: issued now so they land long before their
            # consumers (p3g0/p0g1 fillers)
            nc.sync.dma_start(qf[2][:], qT[:, 2])
            nc.sync.dma_start(qf[3][:], qT[:, 3])
            for dc in range(NDC):
                emit_kproj(0, dc)
            # kt0's slot is consumed; stream kt2/kt3 (consumed ~30us later)
            kt_tiles[2] = ktp.tile(
                [P, EC, 512], F16, tag="kt", bufs=2, name="kt_2"
            )
            nc.scalar.dma_start(kt_tiles[2][:], kT[:, 2])
            for dc in range(NDC):
                emit_kproj(1, dc)
            kt_tiles[3] = ktp.tile(
                [P, EC, 512], F16, tag="kt", bufs=2, name="kt_3"
            )
            nc.scalar.dma_start(kt_tiles[3][:], kT[:, 3])
            nc.scalar.dma_start(wo_sb[:], woT[:])
            emit_qproj(0, 0)
            emit_qproj(0, 1)

            # ---- filler schedule for the attention pair loops -------------
            # One unit pops per kj step.  Order respects data dependencies:
            # K lt2/3 and Q lt2/3 finish before the g1 pass touches them,
            # V8-15 before g1's PV reads k>=1024, out-projection for q<1024
            # unlocks after the whole g0 pass.
            fillers = {
                (0, 0): [lambda dc=dc: emit_kproj(2, dc) for dc in range(NDC)]
                + [lambda: emit_qproj(1, 0), lambda: emit_qproj(1, 1)]
                + [lambda lc=lc: emit_vproj(lc) for lc in (8, 9)],
                (1, 0): [lambda dc=dc: emit_kproj(3, dc) for dc in range(NDC)]
                + [lambda: emit_qproj(2, 0), lambda: emit_qproj(2, 1)]
                + [lambda lc=lc: emit_vproj(lc) for lc in (10, 11)],
                (2, 0): [lambda: emit_qproj(3, 0), lambda: emit_qproj(3, 1)]
                + [lambda lc=lc: emit_vproj(lc) for lc in (12, 13)],
                (3, 0): [lambda: emit_qproj(0, 2), lambda: emit_qproj(0, 3)]
                + [lambda lc=lc: emit_vproj(lc) for lc in (14, 15)],
                (0, 1): [
                    lambda dc=dc, lt=lt: emit_qproj(dc, lt)
                    for dc in (1, 2, 3)
                    for lt in (2, 3)
                ],
                (1, 1): [
                    lambda qc=qc, eh=eh: emit_oproj(qc, eh)
                    for qc in (0, 1, 2, 3)
                    for eh in (0, 1)
                ],
                (2, 1): [
                    lambda qc=qc, eh=eh: emit_oproj(qc, eh)
                    for qc in (4, 5, 6, 7)
                    for eh in (0, 1)
                ],
                (3, 1): [],
            }

            # ---- attention: group-interleaved pair order ------------------
            scale = float(1.0 / np.sqrt(DK))
            pending_fin = []
            for grp in range(2):
                for hp in range(NDC):
                    heads = (2 * hp, 2 * hp + 1)
                    filler = fillers[(hp, grp)]
                    glo, ghi = grp * 1024, grp * 1024 + 1024
                    qis = (2 * grp, 2 * grp + 1)
                    n_kj = (8 * grp + 8) if causal else NKC
                    ctx_ps = {
                        (h, qi): psp.tile(
                            [DK + 1, 512],
                            F32,
                            tag="ctx",
                            bufs=4,
                            name=f"ctx_{h}_{grp}_{qi}",
                        )
                        for h in heads
                        for qi in qis
                    }

                    def emit_divide(h, qi):
                        # copy ctx+den out of PSUM at once (frees the bank
                        # for the next group) and launch the denominator
                        # broadcast (DRAM bounce).  The reciprocal+multiply
                        # are DEFERRED several kj steps so the DVE never
                        # queues behind the in-flight bounce.
                        g0 = 64 * (h % 2)
                        cps = ctx_ps[(h, qi)]
                        cpy = small.tile(
                            [DK + 1, 512], F32, tag="cpy", bufs=6,
                            name=f"cpy_{h}_{qi}",
                        )
                        nc.vector.tensor_copy(cpy[:], cps[:])
                        dsc = dsp.tile([1, 512], F32, name=f"dsc_{h}_{qi}", tag="dsc")
                        nc.sync.dma_start(dsc[:], cpy[DK : DK + 1, :])
                        den64 = small.tile(
                            [64, 512], F32, tag="den64", bufs=5,
                            name=f"d64_{h}_{qi}",
                        )
                        nc.sync.dma_start(
                            den64[:], dsc[0:1, :].to_broadcast((64, 512))
                        )
                        hp_ = hp
                        qs = slice(qi * 512, (qi + 1) * 512)

                        def finalize(h=h, qi=qi, hp=hp_, qs=qs, cpy=cpy, den64=den64):
                            g0 = 64 * (h % 2)
                            rec64 = small.tile(
                                [64, 512], F32, tag="rec64", bufs=2,
                                name=f"r64_{h}_{qi}",
                            )
                            nc.vector.reciprocal_approx_fast(rec64[:], den64[:])
                            if g0 == 0:
                                nc.vector.tensor_mul(
                                    CT_sb[0:64, hp, qs], cpy[0:DK, :], rec64[:]
                                )
                            else:
                                tmp = small.tile(
                                    [64, 512], F16, tag="ctmp", bufs=2,
                                    name=f"tmp_{h}_{qi}",
                                )
                                nc.vector.tensor_mul(tmp[:], cpy[0:DK, :], rec64[:])
                                nc.scalar.dma_start(CT_sb[64:128, hp, qs], tmp[:])

                        pending_fin.append(finalize)

                    def emit_st(kj):
                        q0 = max(glo, kj * P) if causal else glo
                        W = ghi - q0
                        base = (q0 // 512) * 512
                        pts = {}
                        for h in heads:
                            g0 = 64 * (h % 2)
                            st = psp.tile(
                                [P, 1024], F32, tag="st", name=f"st_{h}_{grp}_{kj}"
                            )
                            j = q0
                            while j < ghi:
                                w = min(512 - (j % 512), ghi - j)
                                nc.tensor.matmul(
                                    st[:, j - base : j - base + w],
                                    KT_sb[g0 : g0 + 64, hp, kj * P : (kj + 1) * P],
                                    QT_sb[g0 : g0 + 64, hp, j : j + w],
                                    start=True,
                                    stop=True,
                                )
                                j += w
                            pt = ptp.tile(
                                [P, 1024], F16, tag="pt", name=f"pt_{h}_{grp}_{kj}"
                            )
                            nc.scalar.activation(
                                pt[:, q0 - base : q0 - base + W],
                                st[:, q0 - base : q0 - base + W],
                                mybir.ActivationFunctionType.Exp,
                                scale=scale,
                            )
                            if causal and kj * P >= glo:
                                nc.vector.tensor_mul(
                                    pt[:, q0 - base : q0 - base + P],
                                    pt[:, q0 - base : q0 - base + P],
                                    mask_sb,
                                )
                            pts[h] = pt
                        return pts, q0, base

                    def emit_pv(kj, pts):
                        ptd, q0, base = pts
                        for h in heads:
                            pt = ptd[h]
                            for qi in qis:
                                lo = max(qi * 512, q0)
                                w = qi * 512 + 512 - lo
                                if w <= 0:
                                    continue
                                last = (4 * qi + 3) if causal else (NKC - 1)
                                if kj > last:
                                    continue
                                nc.tensor.matmul(
                                    ctx_ps[(h, qi)][:, lo - qi * 512 :],
                                    V_sb[:, kj, h, :],
                                    pt[:, lo - base : lo - base + w],
                                    start=(kj == 0),
                                    stop=(kj == last),
                                )
                                if kj == last:
                                    emit_divide(h, qi)

                    prev = None
                    for kj in range(n_kj):
                        pts = emit_st(kj)
                        if prev is not None:
                            emit_pv(prev[0], prev[1])
                        # finalize an older divide (its bounce has landed)
                        if len(pending_fin) > 4:
                            pending_fin.pop(0)()
                        if filler:
                            filler.pop(0)()
                        prev = (kj, pts)
                    emit_pv(prev[0], prev[1])
                    while filler:
                        filler.pop(0)()

            # flush remaining divide finalizations before the tail reads CT
            while pending_fin:
                pending_fin.pop(0)()

            # ---- output projection tail: q >= 1024 ------------------------
            for qc in range(8, L // P):
                for eh in range(E // 512):
                    emit_oproj(qc, eh)

    nc.compile()
    return nc


def _chunked(x, inner):
    """[outer*inner, n] -> [inner, outer, n] with element [p, c, n] = x[c*inner+p, n]."""
    o = x.shape[0] // inner
    return np.ascontiguousarray(x.reshape(o, inner, *x.shape[1:]).transpose(1, 0, 2))


def _lchunked(x, w):
    """activation [L, E] -> [128, L//w, 8, w] with [p, c, ec, j] =
    x[c*w+j, ec*128+p] (contiguous per-partition staging chunks)."""
    xt = x.T.astype(np.float16)                    # [E, L]
    a = xt.reshape(EC, P, L // w, w)               # [ec, p, c, j]
    return np.ascontiguousarray(a.transpose(1, 2, 0, 3))


def kernel(query, key, value, wq, bq, wk, bk, wv, bv, wo, bo, is_causal):
    return _run(query, key, value, wq, bq, wk, bk, wv, bv, wo, bo, is_causal)[0]


def _run(query, key, value, wq, bq, wk, bk, wv, bv, wo, bo, is_causal, **run_kwargs):
    query = np.asarray(query, dtype=np.float32)
    key = np.asarray(key, dtype=np.float32)
    value = np.asarray(value, dtype=np.float32)
    wq, wk, wv, wo = (np.asarray(w, dtype=np.float32) for w in (wq, wk, wv, wo))
    bq, bk, bv, bo = (np.asarray(b, dtype=np.float32) for b in (bq, bk, bv, bo))
    causal = bool(int(np.asarray(is_causal)))

    if causal not in _BUILT:
        _BUILT[causal] = _build(causal)
    nc = _BUILT[causal]

    # triangular diagonal mask: mask[kp, qf] = 1 if qf >= kp
    qf = np.arange(P)[None, :]
    kp = np.arange(P)[:, None]
    m1 = (qf >= kp).astype(np.float16)                        # [128, 128]

    in_maps = []
    for c in range(NCORES):
        b = c // 2
        g = c % 2
        hs = slice(g * DQ, (g + 1) * DQ)
        cb = np.zeros((P, CB), dtype=np.float16)
        cb[:, 0:P] = m1
        cb[:, P : 2 * P] = m1
        cb[:, 2 * P : 2 * P + DQ] = np.broadcast_to(
            bv[hs].astype(np.float16), (P, DQ)
        )
        cb[:, 2 * P + DQ : CB] = np.broadcast_to(
            (bo if g == 0 else np.zeros_like(bo)).astype(np.float16), (P, E)
        )
        c32 = np.zeros((P, 2 * NDC), dtype=np.float32)
        c32[:, 0:NDC] = bq[hs].reshape(NDC, P).T
        c32[:, NDC : 2 * NDC] = bk[hs].reshape(NDC, P).T
        m = {
            "qT": _lchunked(query[b], 512),
            "kT": _lchunked(key[b], 512),
            "vT": _lchunked(value[b], P),
            "wqT": _chunked(wq[hs, :].T.astype(np.float16), P),
            "wkT": _chunked(wk[hs, :].T.astype(np.float16), P),
            "wvT": _chunked(wv[hs, :].T.astype(np.float16), P),
            "woT": _chunked(wo[:, hs].T.astype(np.float16), P),
            "cb16": cb,
            "cb32": np.ascontiguousarray(c32),
        }
        in_maps.append(m)

    res = run_bass_kernel_spmd(nc, in_maps, core_ids=list(range(NCORES)), **run_kwargs)

    out = np.empty((B, L, E), dtype=np.float32)
    for b in range(B):
        out[b] = res.results[2 * b]["out"].astype(np.float32) + res.results[
            2 * b + 1
        ]["out"].astype(np.float32)
    return out, res
